# revision 20
# baseline (speedup 1.0000x reference)
"""Trainium2 Bass kernel for nn_AttenConv (gnn message passing).

reference:
    score = user_emb @ item_emb.T            # [U, I]
    score = where(adj > 0, score, 0)
    score = softmax(score, axis=1)
    out   = (score @ item_emb) @ attention_weight   # [U, OUT]

Strategy (8 NeuronCores, data-parallel over users):
  - Each core owns U/8 = 1024 users; item_emb / attention_weight replicated.
  - Scores are computed transposed (items on partitions) so the masked
    exp'd scores P_T [128i, U_LOC] feed the aggregation matmul directly.
  - Softmax denominators are dominated by edge scores (sigma=8 -> e^30+),
    so the reference's exp(0)=1 non-edge contributions are ~1e-10 relative
    and masking can happen on the exp side.
  - user_emb is pre-scaled by A = 128/ln2 on the host, so the score
    matmul directly produces A*s in PSUM. 3/4 of chunks then use a fused
    Schraudolph bitcast-exp+mask: ONE DVE scalar_tensor_tensor computes
    int16((A*s + B) * adj) whose bits, read as bf16, equal
    A_e * e^s * adj (~3% on P, ~1.3e-2 end to end). Non-edges multiply
    to exactly 0. adj stays fp8 {0,1} (1 byte).
  - 1/4 of chunks use exact ACT-engine exp (scale=1/A, bias=ln A -> same
    consistent A_e scale, which cancels in the final division), masked on
    the otherwise-idle Pool engine. This keeps ACT, DVE and Pool all
    under the PE's chunk cadence while improving accuracy.
  - adj ships partition-major (16 KiB contiguous per-partition DMA
    descriptors), streamed in groups on the same priority-ordered sync
    queue as the other inputs (small first group so compute starts ~13us).
  - PE stream is software-pipelined: score(c) then agg(c-4), so
    aggregation never waits on the exp/mask chain (keeps the PE
    continuously fed -> HAM k=8 high-activity state -> ~2x matmul rate).
    A short warmup burst reading user_r flips HAM just before the loop.
  - Numerator and denominator come from one matmul against item_aug
    (extra ones column). Division happens after the output projection
    and a PE transpose, as a per-partition tensor_scalar multiply.
  - Score matmuls use fp16 (values fit; ~2^-11 mantissa keeps the
    exp-amplified score error small). P uses bf16 (reaches e^53).
"""

import sys

sys.path.insert(0, "/opt/trn_rl_repo")

import numpy as np
import ml_dtypes

import concourse.bass as bass
import concourse.mybir as mybir
import concourse.tile as tile
from concourse import bacc
from concourse.bass_utils import run_bass_kernel_spmd

U, I, D, OUT = 8192, 16384, 64, 64
NCORES = 8
U_LOC = U // NCORES          # 1024 users per core
NCHUNK = I // 128            # 128 item chunks
NPAIR = NCHUNK // 2
F32 = mybir.dt.float32
F16 = mybir.dt.float16
BF16 = mybir.dt.bfloat16
F8 = mybir.dt.float8e4
I16 = mybir.dt.int16

EXACT_MOD = 3                # chunks with c % EXACT_MOD == 0 use exact exp
GCH = 16                     # max chunks per adj DMA group
# adj DMA groups as (start_chunk, end_chunk): small first groups so the
# pipeline can start early, then full 16-chunk groups.
GROUPS = [(0, 8), (8, 16)] + [(16 * g, 16 * (g + 1)) for g in range(1, 8)]
NPRE_G = 4                   # groups issued in the preamble
AGG_LAG = 6                  # chunks between score(c) and agg(c)
FILL_CHUNKS = 6              # early chunks whose scores run twice (PE filler)

SCH_A = float(np.float32(128.0 / np.log(2.0)))   # folded into user_emb
SCH_C = 4.5                  # Schraudolph mantissa calibration
SCH_B = 16256.0 - SCH_C + 128.0 * float(np.log2(SCH_A))
EXACT_BIAS = float(np.log(SCH_A))   # exp(s + bias) = A_e * e^s
EXACT_SCALE = 1.0 / SCH_A

_cached = {}


def _group_of(c):
    for gi, (cs, ce) in enumerate(GROUPS):
        if cs <= c < ce:
            return gi, cs
    raise ValueError(c)


def build_nc():
    nc = bacc.Bacc("TRN2", target_bir_lowering=False)

    user2_in = nc.dram_tensor("user2", (128, U_LOC), F16, kind="ExternalInput")
    item2_in = nc.dram_tensor("item2", (128, NPAIR * 128), F16, kind="ExternalInput")
    aug2_in = nc.dram_tensor("aug2", (128, NCHUNK * (D + 1)), BF16,
                             kind="ExternalInput")
    w_in = nc.dram_tensor("w", (D, OUT), F32, kind="ExternalInput")
    adjp_in = nc.dram_tensor("adjp", (128, NCHUNK * U_LOC), F8,
                             kind="ExternalInput")
    ident_in = nc.dram_tensor("ident", (128, 128), F32, kind="ExternalInput")
    out = nc.dram_tensor("out", (U_LOC, OUT), F32, kind="ExternalOutput")
    warm_out = nc.dram_tensor("warm_out", (1, 8), F32, kind="ExternalOutput")

    with tile.TileContext(nc) as tc:
        with tc.tile_pool(name="consts", bufs=1) as consts, \
             tc.tile_pool(name="adj", bufs=4) as adj_pool, \
             tc.tile_pool(name="et", bufs=2) as et_pool, \
             tc.tile_pool(name="pt", bufs=7) as pt_pool, \
             tc.tile_pool(name="fin", bufs=1) as fin:

            # ---- preamble: one queue, priority order ----
            adj_tiles = {}

            def issue_adj_group(gi):
                cs, ce = GROUPS[gi]
                t = adj_pool.tile([128, GCH * U_LOC], F8, tag="adjg")
                nc.sync.dma_start(
                    t[:, 0:(ce - cs) * U_LOC],
                    adjp_in[:, cs * U_LOC:ce * U_LOC],
                )
                adj_tiles[gi] = t

            user_r = consts.tile([128, U_LOC], F16, name="user_r")
            nc.sync.dma_start(user_r[:], user2_in[:, :])
            item_r = consts.tile([128, NPAIR * 128], F16, name="item_r")
            nc.sync.dma_start(item_r[:, 0:2048], item2_in[:, 0:2048])
            issue_adj_group(0)
            aug_sb = consts.tile([128, NCHUNK, D + 1], BF16, name="aug_sb")
            aug_r = aug2_in.rearrange("p (c j) -> p c j", j=D + 1)
            nc.sync.dma_start(aug_sb[:, 0:GCH, :], aug_r[:, 0:GCH, :])
            issue_adj_group(1)
            issue_adj_group(2)
            nc.sync.dma_start(aug_sb[:, GCH:NCHUNK, :], aug_r[:, GCH:NCHUNK, :])
            nc.sync.dma_start(item_r[:, 2048:NPAIR * 128],
                              item2_in[:, 2048:NPAIR * 128])
            issue_adj_group(NPRE_G - 1)
            w_sb = consts.tile([D, OUT], F32, name="w_sb")
            nc.sync.dma_start(w_sb[:], w_in[:, :])
            ident = consts.tile([128, 128], F32, name="ident")
            nc.sync.dma_start(ident[:], ident_in[:, :])

            num_sb = consts.tile([D + 1, U_LOC], F32, name="num_sb")
            bias_sb = consts.tile([128, 1], F32, name="bias_sb")
            nc.vector.memset(bias_sb[:], EXACT_BIAS)
            scale_sb = consts.tile([128, 1], F32, name="scale_sb")
            nc.vector.memset(scale_sb[:], EXACT_SCALE)

            # ---- PE warmup burst to flip HAM high-activity mode.
            # Reads user_r so it starts only once real data is landing and
            # the k=8 state persists into the main loop.
            with tc.tile_pool(name="ps_w", bufs=1, space="PSUM") as ps_w:
                warm_ps = ps_w.tile([128, 512], F32, name="warm_ps")
                for _ in range(10):
                    nc.tensor.matmul(warm_ps[:], user_r[:, 0:128],
                                     user_r[:, 0:512], start=True, stop=True)
                wo = consts.tile([1, 8], F32, name="wo")
                nc.vector.tensor_copy(wo[:], warm_ps[0:1, 0:8])
                nc.sync.dma_start(warm_out[:, :], wo[:])

            # ---- main loop, software-pipelined at chunk granularity ----
            # iteration c emits: exp/mask(c-1) | adj prefetch | score(c) |
            # agg(c-AGG_LAG)
            with tc.tile_pool(name="ps_s", bufs=3, space="PSUM") as ps_s, \
                 tc.tile_pool(name="ps_num", bufs=1, space="PSUM") as ps_num:
                num_ps = ps_num.tile([D + 1, U_LOC], F32, name="num_ps")
                s_tiles = {}
                p_tiles = {}

                def emit_score(c):
                    p, e = divmod(c, 2)
                    lo = 64 * e
                    s_t = ps_s.tile([128, U_LOC], F32, tag="s_t")
                    # during pipeline fill there are no agg matmuls yet;
                    # run the score twice (same output) to keep the PE
                    # gapless so the HAM high-activity state holds
                    for _ in range(2 if c < FILL_CHUNKS else 1):
                        for h in range(U_LOC // 512):
                            nc.tensor.matmul(
                                s_t[:, h * 512:(h + 1) * 512],
                                item_r[lo:lo + 64, p * 128:(p + 1) * 128],
                                user_r[lo:lo + 64, h * 512:(h + 1) * 512],
                                start=True, stop=True,
                                perf_mode=mybir.MatmulPerfMode.DoublePixel,
                            )
                    s_tiles[c] = s_t

                def emit_p(c):
                    """P[c] = A_e * e^s * adj, bf16, via fused Schraudolph
                    (DVE) or exact exp (ACT) + mask (Pool)."""
                    gi, cs = _group_of(c)
                    adj_sl = adj_tiles[gi][:, (c - cs) * U_LOC:
                                           (c - cs + 1) * U_LOC]
                    s_t = s_tiles.pop(c)
                    p_t = pt_pool.tile([128, U_LOC], BF16, tag="p_t")
                    if c % EXACT_MOD == 0:
                        e_t = et_pool.tile([128, U_LOC], BF16, tag="e_t")
                        nc.scalar.activation(
                            e_t[:], s_t[:], mybir.ActivationFunctionType.Exp,
                            bias=bias_sb[0:128, 0:1],
                            scale=scale_sb[0:128, 0:1],
                        )
                        nc.gpsimd.tensor_tensor(
                            p_t[:], e_t[:], adj_sl, mybir.AluOpType.mult,
                        )
                    else:
                        nc.vector.scalar_tensor_tensor(
                            p_t[:].bitcast(I16), s_t[:], SCH_B, adj_sl,
                            op0=mybir.AluOpType.add,
                            op1=mybir.AluOpType.mult,
                        )
                    p_tiles[c] = p_t

                def emit_agg(c):
                    p_t = p_tiles.pop(c)
                    for h in range(U_LOC // 512):
                        nc.tensor.matmul(
                            num_ps[:, h * 512:(h + 1) * 512],
                            aug_sb[:, c, :],
                            p_t[:, h * 512:(h + 1) * 512],
                            start=(c == 0), stop=(c == NCHUNK - 1),
                        )

                for c in range(NCHUNK + AGG_LAG):
                    if 0 <= c - 1 < NCHUNK:
                        emit_p(c - 1)
                    if c < NCHUNK:
                        gi, cs = _group_of(c)
                        if c == cs and gi >= NPRE_G - 1 \
                                and gi + 1 < len(GROUPS):
                            issue_adj_group(gi + 1)
                        emit_score(c)
                    if 0 <= c - AGG_LAG:
                        emit_agg(c - AGG_LAG)
                nc.vector.tensor_copy(num_sb[:], num_ps[:])

            # ---- epilogue: projection, transpose, normalize, store ----
            with tc.tile_pool(name="ps_f", bufs=1, space="PSUM") as ps_f, \
                 tc.tile_pool(name="ps_t", bufs=4, space="PSUM") as ps_t:
                proj_ps = ps_f.tile([OUT, U_LOC], F32, name="proj_ps")
                for h in range(U_LOC // 512):
                    nc.tensor.matmul(
                        proj_ps[:, h * 512:(h + 1) * 512],
                        w_sb[:],
                        num_sb[0:D, h * 512:(h + 1) * 512],
                        start=True, stop=True,
                    )
                # comb rows 0:64 = projected numerator, row 64 = denominator;
                # rows 65:127 are never read after the transpose.
                comb = fin.tile([128, U_LOC], F32, name="comb")
                nc.vector.tensor_copy(comb[0:OUT, :], proj_ps[:])
                nc.vector.tensor_copy(comb[OUT:OUT + 1, :], num_sb[D:D + 1, :])
                o_all = fin.tile([128, U_LOC // 128, OUT], F32, name="o_all")
                for t in range(U_LOC // 128):
                    tp = ps_t.tile([128, 128], F32, tag="tp")
                    nc.tensor.transpose(
                        tp[:], comb[:, t * 128:(t + 1) * 128], ident[:]
                    )
                    r_sb = fin.tile([128, 1], F32, tag="r")
                    nc.vector.reciprocal(r_sb[:], tp[:, OUT:OUT + 1])
                    nc.vector.tensor_scalar_mul(
                        o_all[:, t, :], tp[:, 0:OUT], r_sb[:]
                    )
                nc.sync.dma_start(
                    out.rearrange("(t p) o -> p t o", p=128), o_all[:]
                )

    nc.finalize()
    return nc


def prep_inputs(user_emb, item_emb, attention_weight, adj_matrix):
    """Host-side shard + layout prep. Returns per-core input maps."""
    user_emb = np.ascontiguousarray(np.asarray(user_emb, dtype=np.float32))
    item_emb = np.ascontiguousarray(np.asarray(item_emb, dtype=np.float32))
    attention_weight = np.ascontiguousarray(
        np.asarray(attention_weight, dtype=np.float32))
    adj_matrix = np.asarray(adj_matrix)

    item_t = np.ascontiguousarray(item_emb.T)                      # [D, I]
    # chunk-pair stacking: [128, NPAIR*128] — rows 0:64 even chunk,
    # rows 64:128 odd chunk of each pair
    it3 = item_t.reshape(D, NCHUNK, 128)
    item2 = np.concatenate([it3[:, 0::2, :], it3[:, 1::2, :]],
                           axis=0).reshape(128, NPAIR * 128)
    item2 = np.ascontiguousarray(item2.astype(np.float16))

    # item_aug partition-major: aug2[p, c*65+j] = item_aug[c*128+p, j]
    item_aug = np.empty((I, D + 1), dtype=ml_dtypes.bfloat16)
    item_aug[:, :D] = item_emb.astype(ml_dtypes.bfloat16)
    item_aug[:, D] = 1.0
    aug2 = np.ascontiguousarray(
        item_aug.reshape(NCHUNK, 128, D + 1).transpose(1, 0, 2)
        .reshape(128, NCHUNK * (D + 1)))

    in_maps = []
    for cc in range(NCORES):
        lo, hi = cc * U_LOC, (cc + 1) * U_LOC
        # user pre-scaled by A so the score matmul produces A*s directly
        ut = user_emb[lo:hi].T * np.float32(SCH_A)                # [D, U_LOC]
        user2 = np.ascontiguousarray(
            np.concatenate([ut, ut], axis=0).astype(np.float16))
        adjp = np.ascontiguousarray(
            (adj_matrix[lo:hi].T > 0).astype(ml_dtypes.float8_e4m3)
            .reshape(NCHUNK, 128, U_LOC).transpose(1, 0, 2)
            .reshape(128, NCHUNK * U_LOC))
        in_maps.append({
            "user2": user2,
            "item2": item2,
            "aug2": aug2,
            "w": attention_weight,
            "adjp": adjp,
            "ident": np.eye(128, dtype=np.float32),
        })
    return in_maps


def run(in_maps, trace=False, **kw):
    if "nc" not in _cached:
        _cached["nc"] = build_nc()
    return run_bass_kernel_spmd(
        _cached["nc"], in_maps, core_ids=list(range(NCORES)), trace=trace, **kw
    )


def kernel(user_emb, item_emb, attention_weight, adj_matrix):
    in_maps = prep_inputs(user_emb, item_emb, attention_weight, adj_matrix)
    res = run(in_maps)
    return np.concatenate([r["out"] for r in res.results], axis=0)


if __name__ == "__main__":
    rng = np.random.default_rng(0)
    ue = rng.standard_normal((U, D), dtype=np.float32)
    ie = rng.standard_normal((I, D), dtype=np.float32)
    aw = (rng.standard_normal((D, OUT)) / np.sqrt(D)).astype(np.float32)
    adj = rng.integers(0, 2, size=(U, I)).astype(np.int32)
    o = kernel(ue, ie, aw, adj)
    print("out", o.shape, o.dtype, np.abs(o).max())


# revision 28
# speedup vs baseline: 1.0169x; 1.0169x over previous
"""Trainium2 Bass kernel for nn_AttenConv (gnn message passing).

reference:
    score = user_emb @ item_emb.T            # [U, I]
    score = where(adj > 0, score, 0)
    score = softmax(score, axis=1)
    out   = (score @ item_emb) @ attention_weight   # [U, OUT]

Strategy (8 NeuronCores, data-parallel over users):
  - Each core owns U/8 = 1024 users; item_emb / attention_weight replicated.
  - Scores are computed transposed (items on partitions) so the masked
    exp'd scores P_T [128i, U_LOC] feed the aggregation matmul directly.
  - Softmax denominators are dominated by edge scores (sigma=8 -> e^30+),
    so the reference's exp(0)=1 non-edge contributions are ~1e-10 relative
    and masking can happen on the exp side.
  - user_emb is pre-scaled by A = 128/ln2 on the host, so the score
    matmul directly produces A*s in PSUM. 3/4 of chunks then use a fused
    Schraudolph bitcast-exp+mask: ONE DVE scalar_tensor_tensor computes
    int16((A*s + B) * adj) whose bits, read as bf16, equal
    A_e * e^s * adj (~3% on P, ~1.3e-2 end to end). Non-edges multiply
    to exactly 0. adj stays fp8 {0,1} (1 byte).
  - 1/4 of chunks use exact ACT-engine exp (scale=1/A, bias=ln A -> same
    consistent A_e scale, which cancels in the final division), masked on
    the otherwise-idle Pool engine. This keeps ACT, DVE and Pool all
    under the PE's chunk cadence while improving accuracy.
  - adj ships partition-major (16 KiB contiguous per-partition DMA
    descriptors), streamed in groups on the same priority-ordered sync
    queue as the other inputs (small first group so compute starts ~13us).
  - PE stream is software-pipelined: score(c) then agg(c-4), so
    aggregation never waits on the exp/mask chain (keeps the PE
    continuously fed -> HAM k=8 high-activity state -> ~2x matmul rate).
    A short warmup burst reading user_r flips HAM just before the loop.
  - Numerator and denominator come from one matmul against item_aug
    (extra ones column). Division happens after the output projection
    and a PE transpose, as a per-partition tensor_scalar multiply.
  - Score matmuls use fp16 (values fit; ~2^-11 mantissa keeps the
    exp-amplified score error small). P uses bf16 (reaches e^53).
"""

import sys

sys.path.insert(0, "/opt/trn_rl_repo")

import numpy as np
import ml_dtypes

import concourse.bass as bass
import concourse.mybir as mybir
import concourse.tile as tile
from concourse import bacc
from concourse.bass_utils import run_bass_kernel_spmd

U, I, D, OUT = 8192, 16384, 64, 64
NCORES = 8
U_LOC = U // NCORES          # 1024 users per core
NCHUNK = I // 128            # 128 item chunks
NPAIR = NCHUNK // 2
F32 = mybir.dt.float32
F16 = mybir.dt.float16
BF16 = mybir.dt.bfloat16
F8 = mybir.dt.float8e4
I16 = mybir.dt.int16

EXACT_MOD = 3                # chunks with c % EXACT_MOD == 0 use exact exp
GCH = 16                     # max chunks per adj DMA group
# adj DMA groups as (start_chunk, end_chunk): small first groups so the
# pipeline can start early, then full 16-chunk groups.
GROUPS = [(0, 8), (8, 16)] + [(16 * g, 16 * (g + 1)) for g in range(1, 8)]
NPRE_G = 4                   # groups issued in the preamble
AGG_LAG = 6                  # chunks between score(c) and agg(c)
FILL_CHUNKS = 6              # early chunks whose scores run twice (PE filler)

SCH_A = float(np.float32(128.0 / np.log(2.0)))   # folded into user_emb
SCH_C = 4.5                  # Schraudolph mantissa calibration
SCH_B = 16256.0 - SCH_C + 128.0 * float(np.log2(SCH_A))
EXACT_BIAS = float(np.log(SCH_A))   # exp(s + bias) = A_e * e^s
EXACT_SCALE = 1.0 / SCH_A

_cached = {}


def _group_of(c):
    for gi, (cs, ce) in enumerate(GROUPS):
        if cs <= c < ce:
            return gi, cs
    raise ValueError(c)


def build_nc():
    nc = bacc.Bacc("TRN2", target_bir_lowering=False)

    user2_in = nc.dram_tensor("user2", (128, U_LOC), F16, kind="ExternalInput")
    item2_in = nc.dram_tensor("item2", (128, NPAIR * 128), F16, kind="ExternalInput")
    aug2_in = nc.dram_tensor("aug2", (128, NCHUNK * (D + 1)), BF16,
                             kind="ExternalInput")
    w_in = nc.dram_tensor("w", (D, OUT), BF16, kind="ExternalInput")
    adjp_in = nc.dram_tensor("adjp", (128, NCHUNK * U_LOC), F8,
                             kind="ExternalInput")
    ident_in = nc.dram_tensor("ident", (128, 128), F32, kind="ExternalInput")
    out = nc.dram_tensor("out", (U_LOC, OUT), F32, kind="ExternalOutput")
    warm_out = nc.dram_tensor("warm_out", (1, 8), F32, kind="ExternalOutput")

    with tile.TileContext(nc) as tc:
        with tc.tile_pool(name="consts", bufs=1) as consts, \
             tc.tile_pool(name="adj", bufs=4) as adj_pool, \
             tc.tile_pool(name="et", bufs=2) as et_pool, \
             tc.tile_pool(name="pt", bufs=7) as pt_pool, \
             tc.tile_pool(name="fin", bufs=1) as fin:

            # ---- preamble: one queue, priority order ----
            adj_tiles = {}

            def issue_adj_group(gi):
                cs, ce = GROUPS[gi]
                t = adj_pool.tile([128, GCH * U_LOC], F8, tag="adjg")
                nc.sync.dma_start(
                    t[:, 0:(ce - cs) * U_LOC],
                    adjp_in[:, cs * U_LOC:ce * U_LOC],
                )
                adj_tiles[gi] = t

            user_r = consts.tile([128, U_LOC], F16, name="user_r")
            nc.sync.dma_start(user_r[:], user2_in[:, :])
            item_r = consts.tile([128, NPAIR * 128], F16, name="item_r")
            nc.sync.dma_start(item_r[:, 0:2048], item2_in[:, 0:2048])
            issue_adj_group(0)
            aug_sb = consts.tile([128, NCHUNK, D + 1], BF16, name="aug_sb")
            aug_r = aug2_in.rearrange("p (c j) -> p c j", j=D + 1)
            nc.sync.dma_start(aug_sb[:, 0:GCH, :], aug_r[:, 0:GCH, :])
            issue_adj_group(1)
            issue_adj_group(2)
            nc.sync.dma_start(aug_sb[:, GCH:NCHUNK, :], aug_r[:, GCH:NCHUNK, :])
            nc.sync.dma_start(item_r[:, 2048:NPAIR * 128],
                              item2_in[:, 2048:NPAIR * 128])
            issue_adj_group(NPRE_G - 1)
            w_sb = consts.tile([D, OUT], BF16, name="w_sb")
            nc.sync.dma_start(w_sb[:], w_in[:, :])
            ident = consts.tile([128, 128], F32, name="ident")
            nc.sync.dma_start(ident[:], ident_in[:, :])

            num_sb = consts.tile([D + 1, U_LOC], BF16, name="num_sb")
            bias_sb = consts.tile([128, 1], F32, name="bias_sb")
            nc.vector.memset(bias_sb[:], EXACT_BIAS)
            scale_sb = consts.tile([128, 1], F32, name="scale_sb")
            nc.vector.memset(scale_sb[:], EXACT_SCALE)

            # ---- PE warmup burst to flip HAM high-activity mode.
            # Reads user_r so it starts only once real data is landing and
            # the k=8 state persists into the main loop.
            with tc.tile_pool(name="ps_w", bufs=1, space="PSUM") as ps_w:
                warm_ps = ps_w.tile([128, 512], F32, name="warm_ps")
                for _ in range(10):
                    nc.tensor.matmul(warm_ps[:], user_r[:, 0:128],
                                     user_r[:, 0:512], start=True, stop=True)
                wo = consts.tile([1, 8], F32, name="wo")
                nc.vector.tensor_copy(wo[:], warm_ps[0:1, 0:8])
                nc.sync.dma_start(warm_out[:, :], wo[:])

            # ---- main loop, software-pipelined at chunk granularity ----
            # iteration c emits: exp/mask(c-1) | adj prefetch | score(c) |
            # agg(c-AGG_LAG)
            with tc.tile_pool(name="ps_num", bufs=1, space="PSUM") as ps_num:
              num_ps = ps_num.tile([D + 1, U_LOC], F32, name="num_ps")
              with tc.tile_pool(name="ps_s", bufs=3, space="PSUM") as ps_s:
                s_tiles = {}
                p_tiles = {}

                def emit_score(c):
                    p, e = divmod(c, 2)
                    lo = 64 * e
                    s_t = ps_s.tile([128, U_LOC], F32, tag="s_t")
                    # during pipeline fill there are no agg matmuls yet;
                    # run the score twice (same output) to keep the PE
                    # gapless so the HAM high-activity state holds
                    for _ in range(2 if c < FILL_CHUNKS else 1):
                        for h in range(U_LOC // 512):
                            nc.tensor.matmul(
                                s_t[:, h * 512:(h + 1) * 512],
                                item_r[lo:lo + 64, p * 128:(p + 1) * 128],
                                user_r[lo:lo + 64, h * 512:(h + 1) * 512],
                                start=True, stop=True,
                            )
                    s_tiles[c] = s_t

                def emit_p(c):
                    """P[c] = A_e * e^s * adj, bf16, via fused Schraudolph
                    (DVE) or exact exp (ACT) + mask (Pool)."""
                    gi, cs = _group_of(c)
                    adj_sl = adj_tiles[gi][:, (c - cs) * U_LOC:
                                           (c - cs + 1) * U_LOC]
                    s_t = s_tiles.pop(c)
                    p_t = pt_pool.tile([128, U_LOC], BF16, tag="p_t")
                    if c % EXACT_MOD == 0:
                        e_t = et_pool.tile([128, U_LOC], BF16, tag="e_t")
                        nc.scalar.activation(
                            e_t[:], s_t[:], mybir.ActivationFunctionType.Exp,
                            bias=bias_sb[0:128, 0:1],
                            scale=scale_sb[0:128, 0:1],
                        )
                        nc.gpsimd.tensor_tensor(
                            p_t[:], e_t[:], adj_sl, mybir.AluOpType.mult,
                        )
                    else:
                        nc.vector.scalar_tensor_tensor(
                            p_t[:].bitcast(I16), s_t[:], SCH_B, adj_sl,
                            op0=mybir.AluOpType.add,
                            op1=mybir.AluOpType.mult,
                        )
                    p_tiles[c] = p_t

                def emit_agg(c):
                    p_t = p_tiles.pop(c)
                    for h in range(U_LOC // 512):
                        nc.tensor.matmul(
                            num_ps[:, h * 512:(h + 1) * 512],
                            aug_sb[:, c, :],
                            p_t[:, h * 512:(h + 1) * 512],
                            start=(c == 0), stop=(c == NCHUNK - 1),
                        )

                for c in range(NCHUNK + AGG_LAG):
                    if 0 <= c - 1 < NCHUNK:
                        emit_p(c - 1)
                    if c < NCHUNK:
                        gi, cs = _group_of(c)
                        if c == cs and gi >= NPRE_G - 1 \
                                and gi + 1 < len(GROUPS):
                            issue_adj_group(gi + 1)
                        emit_score(c)
                    if 0 <= c - AGG_LAG:
                        emit_agg(c - AGG_LAG)

              # ---- epilogue: projection, transpose, normalize, store ----
              # num copy halves interleave with the projection matmuls
              with tc.tile_pool(name="ps_f", bufs=1, space="PSUM") as ps_f, \
                   tc.tile_pool(name="ps_t", bufs=4, space="PSUM") as ps_t:
                proj_ps = ps_f.tile([OUT, U_LOC], F32, name="proj_ps")
                for h in range(U_LOC // 512):
                    nc.vector.tensor_copy(num_sb[:, h * 512:(h + 1) * 512],
                                          num_ps[:, h * 512:(h + 1) * 512])
                    nc.tensor.matmul(
                        proj_ps[:, h * 512:(h + 1) * 512],
                        w_sb[:],
                        num_sb[0:D, h * 512:(h + 1) * 512],
                        start=True, stop=True,
                    )
                # comb rows 0:64 = projected numerator, row 64 = denominator;
                # rows 65:127 are never read after the transpose.
                comb = fin.tile([128, U_LOC], F32, name="comb")
                nc.vector.tensor_copy(comb[0:OUT, :], proj_ps[:])
                nc.vector.tensor_copy(comb[OUT:OUT + 1, :], num_sb[D:D + 1, :])
                o_all = fin.tile([128, U_LOC // 128, OUT], F32, name="o_all")
                for t in range(U_LOC // 128):
                    tp = ps_t.tile([128, 128], F32, tag="tp")
                    nc.tensor.transpose(
                        tp[:], comb[:, t * 128:(t + 1) * 128], ident[:]
                    )
                    r_sb = fin.tile([128, 1], F32, tag="r")
                    nc.vector.reciprocal(r_sb[:], tp[:, OUT:OUT + 1])
                    nc.vector.tensor_scalar_mul(
                        o_all[:, t, :], tp[:, 0:OUT], r_sb[:]
                    )
                nc.sync.dma_start(
                    out.rearrange("(t p) o -> p t o", p=128), o_all[:]
                )

    nc.finalize()
    return nc


def prep_inputs(user_emb, item_emb, attention_weight, adj_matrix):
    """Host-side shard + layout prep. Returns per-core input maps."""
    user_emb = np.ascontiguousarray(np.asarray(user_emb, dtype=np.float32))
    item_emb = np.ascontiguousarray(np.asarray(item_emb, dtype=np.float32))
    attention_weight = np.ascontiguousarray(
        np.asarray(attention_weight, dtype=np.float32))
    adj_matrix = np.asarray(adj_matrix)

    item_t = np.ascontiguousarray(item_emb.T)                      # [D, I]
    # chunk-pair stacking: [128, NPAIR*128] — rows 0:64 even chunk,
    # rows 64:128 odd chunk of each pair
    it3 = item_t.reshape(D, NCHUNK, 128)
    item2 = np.concatenate([it3[:, 0::2, :], it3[:, 1::2, :]],
                           axis=0).reshape(128, NPAIR * 128)
    item2 = np.ascontiguousarray(item2.astype(np.float16))

    # item_aug partition-major: aug2[p, c*65+j] = item_aug[c*128+p, j]
    item_aug = np.empty((I, D + 1), dtype=ml_dtypes.bfloat16)
    item_aug[:, :D] = item_emb.astype(ml_dtypes.bfloat16)
    item_aug[:, D] = 1.0
    aug2 = np.ascontiguousarray(
        item_aug.reshape(NCHUNK, 128, D + 1).transpose(1, 0, 2)
        .reshape(128, NCHUNK * (D + 1)))

    in_maps = []
    for cc in range(NCORES):
        lo, hi = cc * U_LOC, (cc + 1) * U_LOC
        # user pre-scaled by A so the score matmul produces A*s directly
        ut = user_emb[lo:hi].T * np.float32(SCH_A)                # [D, U_LOC]
        user2 = np.ascontiguousarray(
            np.concatenate([ut, ut], axis=0).astype(np.float16))
        adjp = np.ascontiguousarray(
            (adj_matrix[lo:hi].T > 0).astype(ml_dtypes.float8_e4m3)
            .reshape(NCHUNK, 128, U_LOC).transpose(1, 0, 2)
            .reshape(128, NCHUNK * U_LOC))
        in_maps.append({
            "user2": user2,
            "item2": item2,
            "aug2": aug2,
            "w": attention_weight.astype(ml_dtypes.bfloat16),
            "adjp": adjp,
            "ident": np.eye(128, dtype=np.float32),
        })
    return in_maps


def run(in_maps, trace=False, **kw):
    if "nc" not in _cached:
        _cached["nc"] = build_nc()
    return run_bass_kernel_spmd(
        _cached["nc"], in_maps, core_ids=list(range(NCORES)), trace=trace, **kw
    )


def kernel(user_emb, item_emb, attention_weight, adj_matrix):
    in_maps = prep_inputs(user_emb, item_emb, attention_weight, adj_matrix)
    res = run(in_maps)
    return np.concatenate([r["out"] for r in res.results], axis=0)


if __name__ == "__main__":
    rng = np.random.default_rng(0)
    ue = rng.standard_normal((U, D), dtype=np.float32)
    ie = rng.standard_normal((I, D), dtype=np.float32)
    aw = (rng.standard_normal((D, OUT)) / np.sqrt(D)).astype(np.float32)
    adj = rng.integers(0, 2, size=(U, I)).astype(np.int32)
    o = kernel(ue, ie, aw, adj)
    print("out", o.shape, o.dtype, np.abs(o).max())
